# revision 1
# baseline (speedup 1.0000x reference)
"""DefectAwareAttention Trainium2 kernel (8-core SPMD), v2.

Strategy: destination-sorted edge processing (as v1) with a rebalanced,
software-pipelined device program driven by cost-model evidence:
v1 was DVE-bound (~700us busy of 840us span) with HWDGE (DMA instruction
ring) second.  v2:

  - Host ships both one-hot matrices (oh: [edge,dstloc], ohT: [dstloc,edge])
    as bf16 in one packed blob per 16 groups (pure index movement, same
    bytes as v1's broadcast dst rows), removing all DVE one-hot generation.
  - All per-edge elementwise work batched to [128, 512] supertile ops:
    DVE does only qk-product, score-bias add, msg-scale, and tiny
    per-window normalize ops.  PSUM->SBUF copies run on ACT (Copy is in
    every activation table set, so no table switches).  The per-head
    segmented reduce runs on GpSimd (otherwise idle).
  - Score bias (geo MLP + host-folded linear-bias cross terms) enters PSUM
    in phase B1 via a rank-4 f32 matmul (lhsT=bias^T, rhs=I4) accumulated
    onto the g2 matmul -- no DVE adds, no DMA-to-PSUM.
  - The +bv contribution (denominator-gated) and +bo fold into the output
    projection as a rank-2 accumulated matmul with host ind/ones rows:
    out^T += [bv@Wo ; bo]^T @ [ind ; 1].
  - Explicit 2-supertile software pipeline skew between the score/message
    front-end and the aggregation/window back-end keeps each in-order
    engine queue free of long cross-engine back-edges.
  - DMA batched: one blob DMA per 16 groups, xtk/ind loaded once, output
    staged 4 windows per DMA.  (HWDGE ring occupancy ~625ns per DMA op.)

Phases per repeat (ACT table constraint: Silu and Exp never share a set):
  B1: geo MLP for all edges (Silu) + bias fold -> DRAM spill [*, 128, 64].
  B2: scores (exp), messages, aggregation, normalize, output projection.
"""
import sys

for _p in ("/opt/trn_rl_repo",):
    if _p not in sys.path:
        sys.path.insert(0, _p)

from contextlib import ExitStack
from dataclasses import dataclass

import numpy as np
import ml_dtypes

import concourse.bass as bass
import concourse.tile as tile
from concourse import bacc, mybir
from concourse.masks import make_identity

BF16 = ml_dtypes.bfloat16
F32 = np.float32

HIDDEN = 128
HEADS = 4
HD = HIDDEN // HEADS
RBF = 40
P = 128          # partitions / window node count / group edge count
NG = 4           # groups per supertile (512 edges)
GB = 4           # supertiles per meta batch (2048 edges)
ST_E = NG * P    # 512 edges per supertile
GB_E = GB * ST_E  # 2048 edges per meta batch
GW = HIDDEN + HEADS  # 132: aggregation width per group (msg || e)
SKEW = 2         # supertile skew between front (scores) and back (agg)


@dataclass
class Cfg:
    n_nodes: int
    n_edges: int
    n_cores: int

    @property
    def n_windows(self):  # global 128-node windows, padded to n_cores multiple
        return -(--(-self.n_nodes // P) // self.n_cores) * self.n_cores

    @property
    def pw(self):  # windows per core
        return self.n_windows // self.n_cores

    @property
    def npad(self):
        return self.n_windows * P


# ----------------------------------------------------------------------------
# device program
# ----------------------------------------------------------------------------

def build_program(cfg: Cfg, G_sched, repeat=1, silu_func=None):
    dt = mybir.dt
    pw = cfg.pw
    T_g = sum(G_sched)
    assert T_g % (NG * GB) == 0
    T_s = T_g // NG
    n_gb = T_s // GB

    g_slot, g_first, g_last = [], [], []
    for k, Gk in enumerate(G_sched):
        for i in range(Gk):
            g_slot.append(k)
            g_first.append(i == 0)
            g_last.append(i == Gk - 1)

    # supertile at which each slot's first group appears (for kw prefetch)
    slot_first_st = {}
    for g, k in enumerate(g_slot):
        if g_first[g]:
            slot_first_st[k] = g // NG

    nc = bacc.Bacc("TRN2", target_bir_lowering=False, debug=False,
                   num_devices=cfg.n_cores)

    ein = lambda n, s, d: nc.dram_tensor(n, s, d, kind="ExternalInput").ap()
    wq_d = ein("Wq", [P, P], dt.bfloat16)        # pre-scaled 1/sqrt(HD)
    wk_d = ein("Wk", [P, P], dt.bfloat16)
    wv_d = ein("Wv", [P, P], dt.bfloat16)
    wo_d = ein("Wo", [P, P], dt.bfloat16)
    wg1_d = ein("Wg1", [RBF, P], dt.bfloat16)
    wg2_d = ein("Wg2", [P, HEADS], dt.bfloat16)
    bg1_d = ein("bg1_col", [P, 1], dt.float32)
    i4_d = ein("ident4", [HEADS, HEADS], dt.float32)
    bvwo_d = ein("bvwo_bo", [2, P], dt.float32)
    n_gb2 = -(-n_gb // 2)
    n_gb4 = -(-n_gb // 4)
    blob_d = ein("blob", [n_gb, P, 3 * GB_E], dt.bfloat16)
    rbf_d = ein("rbfT", [n_gb2, RBF, 2 * GB_E], dt.bfloat16)
    biasT_d = ein("biasT", [n_gb4, HEADS, 4 * GB_E], dt.float32)
    xtk_d = ein("xtk_all", [P, pw * P], dt.bfloat16)
    indc_d = ein("indbar_col", [P, pw], dt.float32)
    indo_d = ein("ind_ones", [2, pw * P], dt.float32)

    spill_d = nc.dram_tensor("geo2", [n_gb2, P, 2 * GB * NG * HEADS],
                             dt.float32).ap()
    out_d = nc.dram_tensor("outT", [P, pw * P], dt.float32,
                           kind="ExternalOutput").ap()

    EXP = mybir.ActivationFunctionType.Exp
    SILU = silu_func or mybir.ActivationFunctionType.Silu

    with tile.TileContext(nc) as tc, ExitStack() as top:
        cpool = top.enter_context(tc.tile_pool(name="consts", bufs=1))
        wq_t = cpool.tile([P, P], dt.bfloat16, tag="wq")
        wk_t = cpool.tile([P, P], dt.bfloat16, tag="wk")
        wv_t = cpool.tile([P, P], dt.bfloat16, tag="wv")
        wo_t = cpool.tile([P, P], dt.bfloat16, tag="wo")
        wg1_t = cpool.tile([RBF, P], dt.bfloat16, tag="wg1")
        wg2_t = cpool.tile([P, HEADS], dt.bfloat16, tag="wg2")
        bg1_t = cpool.tile([P, 1], dt.float32, tag="bg1")
        i4_t = cpool.tile([HEADS, HEADS], dt.float32, tag="i4")
        bvwo_t = cpool.tile([2, P], dt.float32, tag="bvwo")
        xtk_t = cpool.tile([P, pw * P], dt.bfloat16, tag="xtka")
        indc_t = cpool.tile([P, pw], dt.float32, tag="indc")
        indo_t = cpool.tile([2, pw * P], dt.float32, tag="indo")
        ident_t = cpool.tile([P, P], dt.bfloat16, tag="ident")
        for t, d in [(wq_t, wq_d), (wk_t, wk_d), (wv_t, wv_d), (wo_t, wo_d),
                     (wg1_t, wg1_d), (wg2_t, wg2_d), (bg1_t, bg1_d),
                     (i4_t, i4_d), (bvwo_t, bvwo_d), (xtk_t, xtk_d),
                     (indc_t, indc_d), (indo_t, indo_d)]:
            nc.sync.dma_start(t[:], d[:])
        make_identity(nc, ident_t)

        for _rep in range(repeat):
            # ---------------- Phase B1: geo MLP + bias fold ----------------
            with ExitStack() as ph:
                rp = ph.enter_context(tc.tile_pool(name=f"b1r{_rep}", bufs=4))
                bp = ph.enter_context(tc.tile_pool(name=f"b1b{_rep}", bufs=2))
                sp = ph.enter_context(tc.tile_pool(name=f"b1s{_rep}", bufs=5))
                stp = ph.enter_context(tc.tile_pool(name=f"b1st{_rep}", bufs=3))
                g1p = ph.enter_context(tc.tile_pool(name=f"b1g1{_rep}", bufs=2,
                                                    space="PSUM"))
                g2p = ph.enter_context(tc.tile_pool(name=f"b1g2{_rep}", bufs=2,
                                                    space="PSUM"))
                silu_h, stage_h = {}, {}
                rbf_h, bT_h2 = {}, {}
                # 2-supertile (1024-edge) batches; 2-batch skew so the PE
                # never parks g2 pairs (waiting on silu) ahead of later g1s.
                # Input DMAs are batched (rbf: 2 gbs, bias: 4 gbs, spill out:
                # 2 gbs) and prefetched ~2 gbs ahead to hide the ~2.6us
                # HWDGE issue->sem-landing latency and the 625ns/DMA ring.
                B1SKEW = 2

                def b1_fetch(gb):
                    if gb < n_gb and gb % 2 == 0:
                        rbft = rp.tile([RBF, 2 * GB_E], dt.bfloat16,
                                       tag="rbf")
                        nc.sync.dma_start(rbft[:], rbf_d[gb // 2])
                        rbf_h[gb // 2] = rbft
                    if gb < n_gb and gb % 4 == 0:
                        biasT = bp.tile([HEADS, 4 * GB_E], dt.float32,
                                        tag="bT")
                        nc.sync.dma_start(biasT[:], biasT_d[gb // 4])
                        bT_h2[gb // 4] = biasT

                for gb in range(4):
                    b1_fetch(gb)
                n_sb = T_s // 2
                for s2 in range(n_sb + B1SKEW):
                    if s2 < n_sb:
                        s = 2 * s2
                        st = s % GB  # 0 or 2
                        if st == 0:
                            b1_fetch(s // GB + 4)
                        gb = s // GB
                        rbft = rbf_h[gb // 2]
                        r0 = (gb % 2) * GB_E + st * ST_E
                        g1_ps = g1p.tile([P, 2 * ST_E], dt.float32, tag="g1",
                                         space="PSUM")
                        # matmul free dim caps at 512 (one PSUM bank)
                        for h in range(2):
                            nc.tensor.matmul(
                                g1_ps[:, h * ST_E:(h + 1) * ST_E],
                                lhsT=wg1_t[:],
                                rhs=rbft[:, r0 + h * ST_E:r0 + (h + 1) * ST_E],
                                start=True, stop=True)
                        silu = sp.tile([P, 2 * ST_E], dt.bfloat16, tag="silu")
                        nc.scalar.activation(silu[:], g1_ps[:], SILU,
                                             bias=bg1_t[:])
                        silu_h[s2] = silu
                    sb2 = s2 - B1SKEW
                    if sb2 < 0:
                        continue
                    sb = 2 * sb2
                    st = sb % GB
                    gb = sb // GB
                    silu = silu_h.pop(sb2)
                    biasT_b = bT_h2[gb // 4]
                    if st == 0 and gb % 2 == 0:
                        stage = stp.tile([P, 2 * GB * NG * HEADS], dt.float32,
                                         tag="stg")
                        stage_h[gb // 2] = stage
                    stage = stage_h[gb // 2]
                    g2_ps = g2p.tile([P, 2 * NG * HEADS], dt.float32, tag="g2",
                                     space="PSUM")
                    for j in range(2 * NG):
                        o = g2_ps[:, j * HEADS:(j + 1) * HEADS]
                        nc.tensor.matmul(o, lhsT=silu[:, j * P:(j + 1) * P],
                                         rhs=wg2_t[:], start=True, stop=False)
                        c0 = (gb % 4) * GB_E + st * ST_E + j * P
                        nc.tensor.matmul(o, lhsT=biasT_b[:, c0:c0 + P],
                                         rhs=i4_t[:], start=False, stop=True)
                    s0 = (gb % 2) * GB * NG * HEADS + st * NG * HEADS
                    nc.vector.tensor_copy(
                        stage[:, s0:s0 + 2 * NG * HEADS], g2_ps[:])
                    last_batch = (sb2 == n_sb - 1)
                    if (st == GB - 2 and gb % 2 == 1) or last_batch:
                        stage_t = stage_h.pop(gb // 2)
                        # odd-tail pair: only the first 64-col half was
                        # written; spill just that (avoids reading the
                        # stale half of the pool slot)
                        w = 128 if gb % 2 == 1 else 64
                        nc.sync.dma_start(spill_d[gb // 2][:, 0:w],
                                          stage_t[:, 0:w])
                    if st == GB - 2 and gb % 2 == 1:
                        rbf_h.pop(gb // 2, None)
                    if st == GB - 2 and gb % 4 == 3:
                        bT_h2.pop(gb // 4, None)

            # ---------------- Phase B2: scores + aggregation ----------------
            with ExitStack() as ph:
                blp = ph.enter_context(tc.tile_pool(name=f"b2bl{_rep}", bufs=4))
                gp = ph.enter_context(tc.tile_pool(name=f"b2g{_rep}", bufs=3))
                kep = ph.enter_context(tc.tile_pool(name=f"b2ke{_rep}", bufs=3))
                qkp_ = ph.enter_context(tc.tile_pool(name=f"b2qk{_rep}", bufs=3))
                scp = ph.enter_context(tc.tile_pool(name=f"b2sc{_rep}", bufs=3))
                ep = ph.enter_context(tc.tile_pool(name=f"b2e{_rep}",
                                                   bufs=SKEW + 2))
                msp = ph.enter_context(tc.tile_pool(name=f"b2ms{_rep}",
                                                    bufs=SKEW + 2))
                kwp = ph.enter_context(tc.tile_pool(name=f"b2kw{_rep}", bufs=3))
                fp = ph.enter_context(tc.tile_pool(name=f"b2f{_rep}", bufs=2))
                osp = ph.enter_context(tc.tile_pool(name=f"b2os{_rep}", bufs=2))
                qp = ph.enter_context(tc.tile_pool(name=f"b2q{_rep}", bufs=2,
                                                   space="PSUM"))
                kpp = ph.enter_context(tc.tile_pool(name=f"b2kp{_rep}", bufs=1,
                                                    space="PSUM"))
                vp = ph.enter_context(tc.tile_pool(name=f"b2v{_rep}",
                                                   bufs=SKEW,
                                                   space="PSUM"))
                Spool = ph.enter_context(tc.tile_pool(name=f"b2S{_rep}",
                                                      bufs=2, space="PSUM"))
                wpp = ph.enter_context(tc.tile_pool(name=f"b2wp{_rep}", bufs=1,
                                                    space="PSUM"))

                # rotating state between pipeline stages
                sc_h = {}      # s -> (sc tile, gb, st, v_ps)
                msg_h = {}     # s -> msg tile [P, ST_E] bf16
                blob_h = {}    # gb -> blob tile
                geo_h = {}     # gb2 -> geo tile
                kw_h = {}      # slot -> kw_sb tile
                S_ps = None
                out_stage = None
                n_out = 0

                def kw_chain(k):
                    kw_ps = wpp.tile([P, P], dt.float32, tag="wps",
                                     space="PSUM")
                    nc.tensor.matmul(kw_ps[:],
                                     lhsT=xtk_t[:, k * P:(k + 1) * P],
                                     rhs=wk_t[:], start=True, stop=True)
                    kw_sb = kwp.tile([P, P], dt.bfloat16, tag="kw")
                    nc.scalar.copy(kw_sb[:], kw_ps[:])
                    kw_h[k] = kw_sb

                for k in slot_first_st:
                    if slot_first_st[k] == 0:
                        kw_chain(k)

                def b2_fetch(gb):
                    if gb < n_gb:
                        blob = blp.tile([P, 3 * GB_E], dt.bfloat16,
                                        tag="blob")
                        nc.sync.dma_start(blob[:], blob_d[gb])
                        blob_h[gb] = blob
                        if gb % 2 == 0:
                            geo = gp.tile([P, 2 * GB * NG * HEADS],
                                          dt.float32, tag="geo")
                            nc.sync.dma_start(geo[:], spill_d[gb // 2])
                            geo_h[gb // 2] = geo

                b2_fetch(0)
                b2_fetch(1)
                for s in range(T_s + SKEW):
                    # ---------------- mid(s-1): bias add, exp, msg ----------
                    sm = s - 1
                    if 0 <= sm < T_s:
                        sc, gbm, stm, v_ps_m = sc_h.pop(sm)
                        geom = geo_h[gbm // 2]
                        g0 = ((gbm % 2) * GB + stm) * NG * HEADS
                        scb = scp.tile([P, NG * HEADS], dt.float32, tag="scb")
                        nc.gpsimd.tensor_add(
                            scb[:], sc[:], geom[:, g0:g0 + NG * HEADS])
                        # msg layout [P, NG*(128+4)]: per group 128 cols of
                        # e-scaled V then 4 cols of e (exp writes the tail,
                        # the TT reads it back broadcast) -> one PSUM
                        # accumulation group per agg matmul.
                        msg = msp.tile([P, NG * GW], dt.bfloat16, tag="msg")
                        msg_v = msg[:].rearrange("p (g w) -> p g w", w=GW)
                        nc.scalar.activation(
                            msg_v[:, :, HIDDEN:GW],
                            scb[:].rearrange("p (g h) -> p g h", g=NG), EXP)
                        nc.vector.tensor_tensor(
                            out=msg_v[:, :, 0:HIDDEN]
                            .rearrange("p g (h hd) -> p g h hd", hd=HD),
                            in0=v_ps_m[:].rearrange("p (g h hd) -> p g h hd",
                                                    g=NG, hd=HD),
                            in1=msg_v[:, :, HIDDEN:GW]
                            .rearrange("p g (h one) -> p g h one", one=1)
                            .to_broadcast([P, NG, HEADS, HD]),
                            op=mybir.AluOpType.mult)
                        msg_h[sm] = msg

                    # ---------------- front(s) ----------------
                    if s < T_s:
                        st = s % GB
                        gb = s // GB
                        if st == 0:
                            b2_fetch(gb + 2)
                        blob = blob_h[gb]
                        xs0 = st * ST_E
                        ohT0 = GB_E + st * ST_E

                        # kw for windows starting next supertile
                        for k, fs in slot_first_st.items():
                            if fs == s + 1:
                                kw_chain(k)

                        q_ps = qp.tile([P, ST_E], dt.float32, tag="q",
                                       space="PSUM")
                        v_ps = vp.tile([P, ST_E], dt.float32, tag="v",
                                       space="PSUM")
                        ke_ps = kpp.tile([P, ST_E], dt.float32, tag="kps",
                                         space="PSUM")
                        for j in range(NG):
                            xs_j = blob[:, xs0 + j * P:xs0 + (j + 1) * P]
                            o = slice(j * P, (j + 1) * P)
                            nc.tensor.matmul(q_ps[:, o], lhsT=xs_j, rhs=wq_t[:],
                                             start=True, stop=True)
                            nc.tensor.matmul(v_ps[:, o], lhsT=xs_j, rhs=wv_t[:],
                                             start=True, stop=True)
                        for j in range(NG):
                            g = NG * s + j
                            ohT_j = blob[:, ohT0 + j * P:ohT0 + (j + 1) * P]
                            nc.tensor.matmul(ke_ps[:, j * P:(j + 1) * P],
                                             lhsT=ohT_j, rhs=kw_h[g_slot[g]][:],
                                             start=True, stop=True)
                        ke_sb = kep.tile([P, ST_E], dt.bfloat16, tag="keS")
                        nc.scalar.copy(ke_sb[:], ke_ps[:])
                        qk = qkp_.tile([P, ST_E], dt.bfloat16, tag="qk")
                        nc.vector.tensor_mul(qk[:], ke_sb[:], q_ps[:])
                        # bf16 reduce out: DVE accumulates fp32 internally;
                        # only the final per-head score write is bf16, and it
                        # keeps the reduce in the 2x packed DVE mode.
                        sc = scp.tile([P, NG * HEADS], dt.bfloat16, tag="sc")
                        with nc.allow_low_precision(
                                reason="scores downcast post-fp32-accum"):
                            nc.vector.reduce_sum(
                                sc[:],
                                qk[:].rearrange("p (h hd) -> p h hd", hd=HD),
                                axis=mybir.AxisListType.X)
                        sc_h[s] = (sc, gb, st, v_ps)

                    # ---------------- back(s-SKEW): agg + window ends -------
                    sb = s - SKEW
                    if sb < 0:
                        continue
                    msg = msg_h.pop(sb)
                    gbb = sb // GB
                    blob_b = blob_h[gbb]
                    if sb % GB == GB - 1:
                        blob_h.pop(gbb)
                        if gbb % 2 == 1 or gbb == n_gb - 1:
                            geo_h.pop(gbb // 2, None)
                    oh0 = 2 * GB_E + (sb % GB) * ST_E
                    for j in range(NG):
                        g = NG * sb + j
                        if g_first[g]:
                            S_ps = Spool.tile([P, GW], dt.float32,
                                              tag="S", space="PSUM")
                        oh_j = blob_b[:, oh0 + j * P:oh0 + (j + 1) * P]
                        nc.tensor.matmul(S_ps[:], lhsT=oh_j,
                                         rhs=msg[:, j * GW:(j + 1) * GW],
                                         start=g_first[g], stop=g_last[g])
                        if not g_last[g]:
                            continue
                        # ---- window end: normalize + project + emit ----
                        k = g_slot[g]
                        den = fp.tile([P, HEADS], dt.float32, tag="den")
                        nc.vector.tensor_scalar(
                            out=den[:], in0=S_ps[:, HIDDEN:],
                            scalar1=indc_t[:, k:k + 1], scalar2=None,
                            op0=mybir.AluOpType.add)
                        rden = fp.tile([P, HEADS], dt.float32, tag="rden")
                        nc.vector.reciprocal(rden[:], den[:])
                        pnb = fp.tile([P, P], dt.bfloat16, tag="pnb")
                        nc.vector.tensor_tensor(
                            out=pnb[:].rearrange("p (h hd) -> p h hd", hd=HD),
                            in0=S_ps[:, 0:HIDDEN]
                            .rearrange("p (h hd) -> p h hd", hd=HD),
                            in1=rden[:].rearrange("p (h one) -> p h one",
                                                  one=1)
                            .to_broadcast([P, HEADS, HD]),
                            op=mybir.AluOpType.mult)
                        pnT_ps = wpp.tile([P, P], dt.bfloat16, tag="wps",
                                          space="PSUM")
                        nc.tensor.transpose(pnT_ps[:], pnb[:], ident_t[:])
                        pnT = fp.tile([P, P], dt.bfloat16, tag="pnT")
                        nc.scalar.copy(pnT[:], pnT_ps[:])
                        outT_ps = wpp.tile([P, P], dt.float32, tag="wps",
                                           space="PSUM")
                        nc.tensor.matmul(outT_ps[:], lhsT=wo_t[:], rhs=pnT[:],
                                         start=True, stop=False)
                        nc.tensor.matmul(outT_ps[:], lhsT=bvwo_t[:],
                                         rhs=indo_t[:, k * P:(k + 1) * P],
                                         start=False, stop=True)
                        if n_out % 4 == 0:
                            out_stage = osp.tile([P, 4 * P], dt.float32,
                                                 tag="ost")
                        oslot = n_out % 4
                        nc.scalar.copy(
                            out_stage[:, oslot * P:(oslot + 1) * P],
                            outT_ps[:])
                        n_out += 1
                        if oslot == 3 or n_out == pw:
                            o0 = (n_out - 1 - oslot) * P
                            nc.sync.dma_start(
                                out_d[:, o0:o0 + (oslot + 1) * P],
                                out_stage[:, 0:(oslot + 1) * P])

    nc.compile()
    return nc


# ----------------------------------------------------------------------------
# host-side sharding / data prep
# ----------------------------------------------------------------------------

def prep(cfg: Cfg, x, edge_index, edge_attr_rbf, is_defect,
         Wq, bq, Wk, bk, Wv, bv, Wo, bo, Wg1, bg1, Wg2, bg2, defect_bias):
    x = np.asarray(x, F32)
    src = np.asarray(edge_index[0], np.int64)
    dst = np.asarray(edge_index[1], np.int64)
    rbf = np.asarray(edge_attr_rbf, F32)
    dfct = np.asarray(is_defect, np.int64)
    Wq = np.asarray(Wq, F32); bq = np.asarray(bq, F32)
    Wk = np.asarray(Wk, F32); bk = np.asarray(bk, F32)
    Wv = np.asarray(Wv, F32); bv = np.asarray(bv, F32)
    Wo = np.asarray(Wo, F32); bo = np.asarray(bo, F32)
    Wg1 = np.asarray(Wg1, F32); bg1 = np.asarray(bg1, F32)
    Wg2 = np.asarray(Wg2, F32); bg2 = np.asarray(bg2, F32)
    defect_bias = np.asarray(defect_bias, F32)

    scale = 1.0 / np.sqrt(HD)
    Wq_s = Wq * scale
    bq_s = bq * scale
    # bias cross-terms: score = (xWq'+bq')·(xWk+bk) per head
    #   = (xWq')·(xWk) + qb[src] + kb[dst] + cc
    Q0 = x @ Wq_s
    K0 = x @ Wk
    hsl = lambda h: slice(h * HD, (h + 1) * HD)
    qb = np.stack([Q0[:, hsl(h)] @ bk[hsl(h)] for h in range(HEADS)], 1)
    kb = np.stack([K0[:, hsl(h)] @ bq_s[hsl(h)] for h in range(HEADS)], 1)
    cc = np.array([bq_s[hsl(h)] @ bk[hsl(h)] for h in range(HEADS)], F32)
    # defect bias table folded with bg2 and cc: [4 codes, HEADS]
    dtab = defect_bias.T + bg2[None, :] + cc[None, :]

    order = np.argsort(dst, kind="stable")
    src_s, dst_s, rbf_s = src[order], dst[order], rbf[order]
    code_s = dfct[src_s] * 2 + dfct[dst_s]
    bias_eh_s = (dtab[code_s] + qb[src_s] + kb[dst_s]).astype(F32)  # [E,H]

    nw, ncores, pwin = cfg.n_windows, cfg.n_cores, cfg.pw
    bounds = np.searchsorted(dst_s, np.arange(nw + 1) * P)
    wcount = np.diff(bounds)
    wgroups = -(-wcount // P)

    worder = np.argsort(-wgroups, kind="stable")
    core_tot = np.zeros(ncores, np.int64)
    core_wins = [[] for _ in range(ncores)]
    for w in worder:
        cand = [c for c in range(ncores) if len(core_wins[c]) < pwin]
        c = min(cand, key=lambda c: (core_tot[c], len(core_wins[c])))
        core_wins[c].append(w)
        core_tot[c] += wgroups[w]
    G_sched = [max(1, max(wgroups[core_wins[c][k]] for c in range(ncores)))
               for k in range(pwin)]
    pad16 = (-sum(G_sched)) % (NG * GB)
    G_sched[-1] += pad16
    G_sched = [int(g) for g in G_sched]
    T_g = sum(G_sched)
    T_s = T_g // NG
    n_gb = T_s // GB

    xpad = np.zeros((cfg.npad, HIDDEN), F32)
    xpad[:cfg.n_nodes] = x
    nodedeg = np.bincount(dst_s, minlength=cfg.npad)

    i4 = np.eye(HEADS, dtype=F32)
    bvwo_bo = np.stack([bv @ Wo, bo]).astype(F32)

    consts = dict(
        Wq=Wq_s.astype(BF16), Wk=Wk.astype(BF16), Wv=Wv.astype(BF16),
        Wo=Wo.astype(BF16), Wg1=Wg1.astype(BF16), Wg2=Wg2.astype(BF16),
        bg1_col=bg1.reshape(P, 1).copy(),
        ident4=i4, bvwo_bo=bvwo_bo,
    )

    iota = np.arange(P)
    in_maps = []
    for c in range(ncores):
        wins = core_wins[c]
        eids = np.full(T_g * P, -1, np.int64)
        pos = 0
        for k, w in enumerate(wins):
            lo, hi = bounds[w], bounds[w + 1]
            eids[pos:pos + hi - lo] = np.arange(lo, hi)
            pos += G_sched[k] * P
        real = eids >= 0
        e_r = eids[real]

        xsrc_e = np.zeros((T_g * P, HIDDEN), F32)
        dloc = np.full(T_g * P, -1, np.int64)
        beh = np.zeros((T_g * P, HEADS), F32)
        rbf_e = np.zeros((T_g * P, RBF), F32)
        xsrc_e[real] = xpad[src_s[e_r]]
        dloc[real] = dst_s[e_r] % P
        beh[real] = bias_eh_s[e_r]
        rbf_e[real] = rbf_s[e_r]

        # supertile layouts; edge linear order is group-major (j*128 + p)
        dl = dloc.reshape(T_s, NG, P)
        ohT = (dl[:, None, :, :] == iota[None, :, None, None])  # [T_s,128,j,p]
        ohT = ohT.reshape(T_s, P, ST_E).astype(BF16)
        oh = (dl[:, :, :, None] == iota[None, None, None, :])   # [T_s,j,p,128]
        oh = oh.transpose(0, 2, 1, 3).reshape(T_s, P, ST_E).astype(BF16)
        xsT = (xsrc_e.reshape(T_s, ST_E, HIDDEN).transpose(0, 2, 1)
               .astype(BF16))
        blob = np.concatenate(
            [xsT.reshape(n_gb, GB, P, ST_E).transpose(0, 2, 1, 3)
             .reshape(n_gb, P, GB_E),
             ohT.reshape(n_gb, GB, P, ST_E).transpose(0, 2, 1, 3)
             .reshape(n_gb, P, GB_E),
             oh.reshape(n_gb, GB, P, ST_E).transpose(0, 2, 1, 3)
             .reshape(n_gb, P, GB_E)], axis=2)

        n_gb2 = -(-n_gb // 2)
        n_gb4 = -(-n_gb // 4)
        biasT = (beh.reshape(T_s, ST_E, HEADS).transpose(0, 2, 1)
                 .reshape(n_gb, GB, HEADS, ST_E).transpose(0, 2, 1, 3)
                 .reshape(n_gb, HEADS, GB_E).astype(F32))
        biasT4 = np.zeros((n_gb4 * 4, HEADS, GB_E), F32)
        biasT4[:n_gb] = biasT
        biasT4 = (biasT4.reshape(n_gb4, 4, HEADS, GB_E).transpose(0, 2, 1, 3)
                  .reshape(n_gb4, HEADS, 4 * GB_E))
        rbfT = (rbf_e.reshape(T_s, ST_E, RBF).transpose(0, 2, 1)
                .reshape(n_gb, GB, RBF, ST_E).transpose(0, 2, 1, 3)
                .reshape(n_gb, RBF, GB_E).astype(BF16))
        rbfT2 = np.zeros((n_gb2 * 2, RBF, GB_E), BF16)
        rbfT2[:n_gb] = rbfT
        rbfT2 = (rbfT2.reshape(n_gb2, 2, RBF, GB_E).transpose(0, 2, 1, 3)
                 .reshape(n_gb2, RBF, 2 * GB_E))

        xtk_all = np.concatenate(
            [xpad[w * P:(w + 1) * P].T for w in wins], axis=1).astype(BF16)
        ind = np.stack([(nodedeg[w * P:(w + 1) * P] > 0) for w in wins])
        ind = ind.astype(F32)                           # [pw, 128]
        indbar_col = (1.0 - ind).T.copy()               # [128, pw]
        ind_ones = np.stack([ind.reshape(-1),
                             np.ones(pwin * P, F32)]).astype(F32)

        in_maps.append(dict(
            blob=np.ascontiguousarray(blob),
            rbfT=np.ascontiguousarray(rbfT2),
            biasT=np.ascontiguousarray(biasT4),
            xtk_all=np.ascontiguousarray(xtk_all),
            indbar_col=indbar_col,
            ind_ones=ind_ones,
            **consts,
        ))
    return in_maps, core_wins, G_sched


def assemble_output(cfg: Cfg, results, core_wins):
    out = np.zeros((cfg.npad, HIDDEN), F32)
    for c, wins in enumerate(core_wins):
        oT = results[c]["outT"]
        for k, w in enumerate(wins):
            out[w * P:(w + 1) * P] = oT[:, k * P:(k + 1) * P].T
    return out[:cfg.n_nodes]


_CACHE = {}


def _get_program(cfg: Cfg, G_sched):
    key = (cfg.n_nodes, cfg.n_edges, cfg.n_cores, tuple(G_sched))
    if key not in _CACHE:
        _CACHE[key] = build_program(cfg, G_sched)
    return _CACHE[key]


LAST_RESULT = None  # BassKernelResults from the most recent run (for test.py)


def kernel(trace=False, **inputs):
    global LAST_RESULT
    from concourse.bass_utils import run_bass_kernel_spmd
    cfg = Cfg(n_nodes=50000, n_edges=600000, n_cores=8)
    in_maps, core_wins, G_sched = prep(cfg, **inputs)
    nc = _get_program(cfg, G_sched)
    res = run_bass_kernel_spmd(nc, in_maps, core_ids=list(range(cfg.n_cores)),
                               trace=trace)
    LAST_RESULT = res
    return assemble_output(cfg, res.results, core_wins)


# ----------------------------------------------------------------------------
# timing utility (used by test.py; not needed for grading correctness)
# ----------------------------------------------------------------------------

def bench_exec_ns(inputs, iters=7):
    """On-device exec time via program-repeat slope (cancels the ~91 ms axon
    dispatch floor): exec = (wall(R=3) - wall(R=1)) / 2, median over iters."""
    import time
    import jax
    from jax.sharding import Mesh, PartitionSpec, NamedSharding
    from jax.experimental.shard_map import shard_map
    from concourse import bass2jax
    from concourse.bass2jax import _bass_exec_p, install_neuronx_cc_hook
    install_neuronx_cc_hook()

    cfg = Cfg(n_nodes=50000, n_edges=600000, n_cores=8)
    in_maps, core_wins, G_sched = prep(cfg, **inputs)
    n_cores = cfg.n_cores

    def make_runner(nc):
        in_names, out_names, out_avals = [], [], []
        for alloc in nc.m.functions[0].allocations:
            if not isinstance(alloc, mybir.MemoryLocationSet):
                continue
            name = alloc.memorylocations[0].name
            if alloc.kind == "ExternalInput":
                if nc.partition_id_tensor and \
                        name == nc.partition_id_tensor.name:
                    continue
                in_names.append(name)
            elif alloc.kind == "ExternalOutput":
                out_names.append(name)
                out_avals.append(jax.core.ShapedArray(
                    tuple(alloc.tensor_shape), mybir.dt.np(alloc.dtype)))
        n_params, n_outs = len(in_names), len(out_avals)
        all_in = in_names + out_names
        pname = nc.partition_id_tensor.name if nc.partition_id_tensor else None
        if pname:
            all_in.append(pname)

        def _body(*args):
            operands = list(args)
            if pname:
                operands.append(bass2jax.partition_id_tensor())
            return tuple(_bass_exec_p.bind(
                *operands, out_avals=tuple(out_avals),
                in_names=tuple(all_in), out_names=tuple(out_names),
                lowering_input_output_aliases=(),
                sim_require_finite=True, sim_require_nnan=True, nc=nc))

        mesh = Mesh(np.asarray(jax.devices()[:n_cores]), ("core",))
        sharded = jax.jit(
            shard_map(_body, mesh=mesh,
                      in_specs=(PartitionSpec("core"),) * (n_params + n_outs),
                      out_specs=(PartitionSpec("core"),) * n_outs,
                      check_rep=False),
            donate_argnums=tuple(range(n_params, n_params + n_outs)),
            keep_unused=True)
        sh = NamedSharding(mesh, PartitionSpec("core"))
        in_bufs = [jax.device_put(
            np.concatenate([np.asarray(in_maps[c][nm])
                            for c in range(n_cores)], 0), sh)
            for nm in in_names]
        jax.block_until_ready(in_bufs)

        def run():
            zs = [jax.device_put(
                np.zeros((n_cores * a.shape[0], *a.shape[1:]), a.dtype), sh)
                for a in out_avals]
            jax.block_until_ready(zs)
            t0 = time.time()
            jax.block_until_ready(sharded(*in_bufs, *zs))
            return time.time() - t0

        return run

    run1 = make_runner(build_program(cfg, G_sched, repeat=1))
    run3 = make_runner(build_program(cfg, G_sched, repeat=3))
    w1, w3 = [], []
    run1(); run3()  # warm NEFF load
    for _ in range(iters):
        w1.append(run1())
        w3.append(run3())
    exec_s = (float(np.median(w3)) - float(np.median(w1))) / 2
    return max(0, int(exec_s * 1e9))

